# revision 6
# baseline (speedup 1.0000x reference)
"""Trainium2 Bass kernel for the KOSLMSSM dense_mlp problem.

Reference computation (B=64, H=512):
    z = tanh(gates[:, :H]);  M = sigmoid(gates[:, H:])        # [B,H]
    innov[b,i,j]  = z[b,i] - At_prev[b,i,j]*c_prev[b,i]*M[b,i]
    hidden[b,i,k] = relu(innov[b,i,:] @ W1[k,:] + b1[k])      # [B,H,3H]
    K_t[b,i,h]    = hidden[b,i,:] @ W2[h,:] + b2[h]           # [B,H,H]
    A_t   = sigmoid(I - K_t*M) * A
    c_new = A_t*c_prev + K_t*z
    h_new = mean_j(c_new*M)
Returns (h_new, c_new, A_t).

Sharding: data-parallel over batch across 8 NeuronCores (8 batches/core).
"""

import numpy as np

H = 512
B = 64
NCORES = 8
BLOC = B // NCORES          # batches per core
KT = (3 * H) // 128         # 12 k-tiles over the 3H hidden dim
IB = H // 128               # 4 row tiles of 128
JC = H // 128               # 4 contraction chunks of 128

_CACHE = {}


def _build_program(with_b2):
    from contextlib import ExitStack

    import concourse.bacc as bacc
    import concourse.mybir as mybir
    import concourse.tile as tile
    from concourse.masks import make_identity

    f32 = mybir.dt.float32
    bf16 = mybir.dt.bfloat16
    AF = mybir.ActivationFunctionType
    OP = mybir.AluOpType

    nc = bacc.Bacc(
        "TRN2", target_bir_lowering=False, debug=False, num_devices=NCORES
    )

    gates_d = nc.dram_tensor("gates", [BLOC, 2 * H], f32, kind="ExternalInput").ap()
    cprev_d = nc.dram_tensor("c_prev", [BLOC, H, 1], f32, kind="ExternalInput").ap()
    A_d = nc.dram_tensor("A", [BLOC, H, H], f32, kind="ExternalInput").ap()
    At_d = nc.dram_tensor("At_prev", [BLOC, H, H], f32, kind="ExternalInput").ap()
    W1_d = nc.dram_tensor("W1", [3 * H, H], f32, kind="ExternalInput").ap()
    b1_d = nc.dram_tensor("b1", [3 * H], f32, kind="ExternalInput").ap()
    W2_d = nc.dram_tensor("W2", [H, 3 * H], f32, kind="ExternalInput").ap()
    b2_d = nc.dram_tensor("b2", [H], f32, kind="ExternalInput").ap()

    hnew_d = nc.dram_tensor("h_new", [BLOC, H], f32, kind="ExternalOutput").ap()
    cnew_d = nc.dram_tensor("c_new", [BLOC, H, H], f32, kind="ExternalOutput").ap()
    At_o_d = nc.dram_tensor("A_t", [BLOC, H, H], f32, kind="ExternalOutput").ap()

    with tile.TileContext(nc) as tc, ExitStack() as ctx:
        const = ctx.enter_context(tc.tile_pool(name="const", bufs=1))
        wpool = ctx.enter_context(tc.tile_pool(name="wpool", bufs=1))
        stage = ctx.enter_context(tc.tile_pool(name="stage", bufs=6))
        work = ctx.enter_context(tc.tile_pool(name="work", bufs=2))
        ptr = ctx.enter_context(tc.tile_pool(name="ptr", bufs=2, space="PSUM"))
        pmm1 = ctx.enter_context(tc.tile_pool(name="pmm1", bufs=2, space="PSUM"))
        pmm2 = ctx.enter_context(tc.tile_pool(name="pmm2", bufs=2, space="PSUM"))

        # ---------------- constants / small inputs ----------------
        ident_f = const.tile([128, 128], f32, tag="ident_f")
        make_identity(nc, ident_f)
        ident_b = const.tile([128, 128], bf16, tag="ident_b")
        make_identity(nc, ident_b)

        # identity slabs for (I - K_t*M): ones at (p, ib*128+p)
        I_tiles = []
        for ib in range(IB):
            it = const.tile([128, H], f32, tag=f"I{ib}")
            nc.gpsimd.memset(it, 0.0)
            nc.gpsimd.affine_select(
                out=it,
                in_=it,
                compare_op=OP.not_equal,
                fill=1.0,
                base=ib * 128,
                pattern=[[-1, H]],
                channel_multiplier=1,
            )
            I_tiles.append(it)

        # gates -> [p, b, zm, ib]
        g_all = const.tile([128, BLOC, 2, IB], f32, tag="g_all")
        nc.sync.dma_start(
            out=g_all,
            in_=gates_d.rearrange("b (zm ib p) -> p b zm ib", zm=2, ib=IB, p=128),
        )
        z_all = const.tile([128, BLOC, IB], f32, tag="z_all")
        nc.scalar.activation(out=z_all, in_=g_all[:, :, 0, :], func=AF.Tanh)
        m_all = const.tile([128, BLOC, IB], f32, tag="m_all")
        nc.scalar.activation(out=m_all, in_=g_all[:, :, 1, :], func=AF.Sigmoid)

        c_sb = const.tile([128, BLOC, IB], f32, tag="c_sb")
        nc.sync.dma_start(
            out=c_sb,
            in_=cprev_d.rearrange("b (ib p) one -> p b (ib one)", ib=IB, p=128),
        )
        # cmn = -c_prev*M ; mneg = -M ; m512 = M/512
        cmn = const.tile([128, BLOC, IB], f32, tag="cmn")
        nc.vector.scalar_tensor_tensor(
            out=cmn, in0=c_sb, scalar=-1.0, in1=m_all, op0=OP.mult, op1=OP.mult
        )
        mneg = const.tile([128, BLOC, IB], f32, tag="mneg")
        nc.vector.tensor_scalar_mul(mneg, m_all, -1.0)
        m512 = const.tile([128, BLOC, IB], f32, tag="m512")
        nc.vector.tensor_scalar_mul(m512, m_all, 1.0 / H)

        b1_sb = const.tile([128, KT], f32, tag="b1_sb")
        nc.sync.dma_start(out=b1_sb, in_=b1_d.rearrange("(kt p) -> p kt", p=128))

        b2_sb = None
        if with_b2:
            b2_sb = const.tile([128, H], f32, tag="b2_sb")
            nc.sync.dma_start(out=b2_sb, in_=b2_d.to_broadcast((128, H)))

        h_tile = const.tile([128, BLOC, IB], f32, tag="h_tile")

        # ---------------- weight prep: W1T (bf16) ----------------
        # w1t[jc] : [128(j), 3H(k)] bf16 ; block (kt,jc) = W1[kt,jc]^T
        w1t = [wpool.tile([128, 3 * H], bf16, tag=f"w1t{jc}", name=f"w1t{jc}") for jc in range(JC)]
        for jc in range(JC):
            for g in range(KT // 4):
                pt = ptr.tile([128, 512], f32, tag="ptr")
                for q in range(4):
                    kt = g * 4 + q
                    ws = stage.tile([128, 128], f32, tag="wstage")
                    nc.sync.dma_start(
                        out=ws,
                        in_=W1_d[
                            kt * 128 : (kt + 1) * 128, jc * 128 : (jc + 1) * 128
                        ],
                    )
                    nc.tensor.transpose(
                        out=pt[:, q * 128 : (q + 1) * 128], in_=ws, identity=ident_f
                    )
                nc.vector.tensor_copy(
                    w1t[jc][:, g * 512 : (g + 1) * 512], pt
                )

        # ---------------- per-batch main loop ----------------
        # W2T prep is interleaved after batch 0's innov stage (see below) so
        # the PE can start batch-0 transposes/MM1 sooner.
        w2t = [wpool.tile([128, H], bf16, tag=f"w2t{kt}", name=f"w2t{kt}") for kt in range(KT)]

        def emit_w2_prep():
            for kt in range(KT):
                pt = ptr.tile([128, 512], f32, tag="ptr")
                for hb in range(IB):
                    ws = stage.tile([128, 128], f32, tag="wstage")
                    nc.sync.dma_start(
                        out=ws,
                        in_=W2_d[
                            hb * 128 : (hb + 1) * 128, kt * 128 : (kt + 1) * 128
                        ],
                    )
                    nc.tensor.transpose(
                        out=pt[:, hb * 128 : (hb + 1) * 128],
                        in_=ws,
                        identity=ident_f,
                    )
                nc.vector.tensor_copy(w2t[kt], pt)

        for b in range(BLOC):
            # load At_prev rows, compute innov (bf16) in natural layout
            innov = []
            for ib in range(IB):
                at = work.tile([128, H], f32, tag="at", bufs=8)
                nc.sync.dma_start(
                    out=at, in_=At_d[b, ib * 128 : (ib + 1) * 128, :]
                )
                inv = work.tile([128, H], bf16, tag="innov", bufs=8)
                nc.vector.tensor_scalar(
                    out=inv,
                    in0=at,
                    scalar1=cmn[:, b, ib : ib + 1],
                    scalar2=z_all[:, b, ib : ib + 1],
                    op0=OP.mult,
                    op1=OP.add,
                )
                innov.append(inv)

            if b == 0:
                emit_w2_prep()

            # transpose innov -> innovT[jc] : [128(j), H(i)] bf16
            innovT = []
            for jc in range(JC):
                pt = ptr.tile([128, 512], bf16, tag="ptrb")
                for ib in range(IB):
                    nc.tensor.transpose(
                        out=pt[:, ib * 128 : (ib + 1) * 128],
                        in_=innov[ib][:, jc * 128 : (jc + 1) * 128],
                        identity=ident_b,
                    )
                iT = work.tile([128, H], bf16, tag="innovT", bufs=8)
                nc.vector.tensor_copy(iT, pt)
                innovT.append(iT)

            # MM1: hiddenT[kt] = relu(W1T-block^T @ innovT + b1)
            ht = work.tile([128, KT, H], bf16, tag="ht")
            for kt in range(KT):
                ph = pmm1.tile([128, H], f32, tag="pmm1")
                for jc in range(JC):
                    nc.tensor.matmul(
                        ph,
                        lhsT=w1t[jc][:, kt * 128 : (kt + 1) * 128],
                        rhs=innovT[jc],
                        start=(jc == 0),
                        stop=(jc == JC - 1),
                    )
                nc.scalar.activation(
                    out=ht[:, kt, :],
                    in_=ph,
                    func=AF.Relu,
                    bias=b1_sb[:, kt : kt + 1],
                )

            # load A rows (needed in epilogue)
            a_rows = []
            for ib in range(IB):
                ar = work.tile([128, H], f32, tag="arows", bufs=8)
                nc.scalar.dma_start(
                    out=ar, in_=A_d[b, ib * 128 : (ib + 1) * 128, :]
                )
                a_rows.append(ar)

            # MM2 + epilogue per row-block
            for ib in range(IB):
                pk = pmm2.tile([128, H], f32, tag="pmm2")
                for kt in range(KT):
                    nc.tensor.matmul(
                        pk,
                        lhsT=ht[:, kt, ib * 128 : (ib + 1) * 128],
                        rhs=w2t[kt],
                        start=(kt == 0),
                        stop=(kt == KT - 1),
                    )

                kt_src = pk
                if with_b2:
                    kb = work.tile([128, H], f32, tag="kb2")
                    nc.vector.tensor_tensor(
                        out=kb, in0=pk, in1=b2_sb, op=OP.add
                    )
                    kt_src = kb

                # sarg = (K_t * -M) + I
                sarg = work.tile([128, H], f32, tag="sarg")
                nc.vector.scalar_tensor_tensor(
                    out=sarg,
                    in0=kt_src,
                    scalar=mneg[:, b, ib : ib + 1],
                    in1=I_tiles[ib],
                    op0=OP.mult,
                    op1=OP.add,
                )
                sig = work.tile([128, H], f32, tag="sig")
                nc.scalar.activation(out=sig, in_=sarg, func=AF.Sigmoid)
                # tkz = K_t * z
                tkz = work.tile([128, H], f32, tag="tkz")
                nc.scalar.activation(
                    out=tkz,
                    in_=kt_src,
                    func=AF.Copy,
                    scale=z_all[:, b, ib : ib + 1],
                )
                # A_t = sig * A
                a_t = work.tile([128, H], f32, tag="a_t", bufs=4)
                nc.vector.tensor_tensor(out=a_t, in0=sig, in1=a_rows[ib], op=OP.mult)
                nc.gpsimd.dma_start(
                    out=At_o_d[b, ib * 128 : (ib + 1) * 128, :], in_=a_t
                )
                # c_new = A_t*c_prev + tkz  (+ fused row-sum)
                cn = work.tile([128, H], f32, tag="cn", bufs=4)
                rs = work.tile([128, 1], f32, tag="rs", bufs=4)
                nc.vector.scalar_tensor_tensor(
                    out=cn,
                    in0=a_t,
                    scalar=c_sb[:, b, ib : ib + 1],
                    in1=tkz,
                    op0=OP.mult,
                    op1=OP.add,
                    accum_out=rs,
                )
                nc.gpsimd.dma_start(
                    out=cnew_d[b, ib * 128 : (ib + 1) * 128, :], in_=cn
                )
                # h_new = rowsum * M/512
                nc.vector.tensor_scalar(
                    out=h_tile[:, b, ib : ib + 1],
                    in0=rs,
                    scalar1=m512[:, b, ib : ib + 1],
                    scalar2=None,
                    op0=OP.mult,
                )

        nc.sync.dma_start(
            out=hnew_d.rearrange("b (ib p) -> p b ib", ib=IB, p=128), in_=h_tile
        )

    nc.compile()
    return nc


def _get_program(with_b2):
    key = ("prog", bool(with_b2))
    if key not in _CACHE:
        _CACHE[key] = _build_program(with_b2)
    return _CACHE[key]


def kernel(**inputs):
    from concourse.bass_utils import run_bass_kernel_spmd

    gates = np.ascontiguousarray(np.asarray(inputs["gates"], dtype=np.float32))
    c_prev = np.ascontiguousarray(np.asarray(inputs["c_prev"], dtype=np.float32))
    A = np.ascontiguousarray(np.asarray(inputs["A"], dtype=np.float32))
    At_prev = np.ascontiguousarray(np.asarray(inputs["At_prev"], dtype=np.float32))
    W1 = np.ascontiguousarray(np.asarray(inputs["W1"], dtype=np.float32))
    b1 = np.ascontiguousarray(np.asarray(inputs["b1"], dtype=np.float32))
    W2 = np.ascontiguousarray(np.asarray(inputs["W2"], dtype=np.float32))
    b2 = np.ascontiguousarray(np.asarray(inputs["b2"], dtype=np.float32))

    with_b2 = bool(np.any(b2))
    nc = _get_program(with_b2)

    in_maps = []
    for c in range(NCORES):
        s = slice(c * BLOC, (c + 1) * BLOC)
        in_maps.append(
            {
                "gates": gates[s],
                "c_prev": c_prev[s],
                "A": A[s],
                "At_prev": At_prev[s],
                "W1": W1,
                "b1": b1,
                "W2": W2,
                "b2": b2,
            }
        )

    res = run_bass_kernel_spmd(nc, in_maps, core_ids=list(range(NCORES)))
    h_new = np.concatenate([res.results[c]["h_new"] for c in range(NCORES)], axis=0)
    c_new = np.concatenate([res.results[c]["c_new"] for c in range(NCORES)], axis=0)
    A_t = np.concatenate([res.results[c]["A_t"] for c in range(NCORES)], axis=0)
    return (
        h_new.astype(np.float32),
        c_new.astype(np.float32),
        A_t.astype(np.float32),
    )


# revision 13
# speedup vs baseline: 38112.6213x; 38112.6213x over previous
"""Trainium2 Bass kernel for the KOSLMSSM dense_mlp problem.

Reference computation (B=64, H=512):
    z = tanh(gates[:, :H]);  M = sigmoid(gates[:, H:])        # [B,H]
    innov[b,i,j]  = z[b,i] - At_prev[b,i,j]*c_prev[b,i]*M[b,i]
    hidden[b,i,k] = relu(innov[b,i,:] @ W1[k,:] + b1[k])      # [B,H,3H]
    K_t[b,i,h]    = hidden[b,i,:] @ W2[h,:] + b2[h]           # [B,H,H]
    A_t   = sigmoid(I - K_t*M) * A
    c_new = A_t*c_prev + K_t*z
    h_new = mean_j(c_new*M)
Returns (h_new, c_new, A_t).

Sharding: data-parallel over batch across 8 NeuronCores (8 batches/core).
"""

import numpy as np

H = 512
B = 64
NCORES = 8
BLOC = B // NCORES          # batches per core
KT = (3 * H) // 128         # 12 k-tiles over the 3H hidden dim
IB = H // 128               # 4 row tiles of 128
JC = H // 128               # 4 contraction chunks of 128

_CACHE = {}


def _build_program(with_b2):
    from contextlib import ExitStack

    import concourse.bacc as bacc
    import concourse.mybir as mybir
    import concourse.tile as tile
    from concourse.masks import make_identity

    f32 = mybir.dt.float32
    bf16 = mybir.dt.bfloat16
    AF = mybir.ActivationFunctionType
    OP = mybir.AluOpType

    nc = bacc.Bacc(
        "TRN2", target_bir_lowering=False, debug=False, num_devices=NCORES
    )

    gates_d = nc.dram_tensor("gates", [BLOC, 2 * H], f32, kind="ExternalInput").ap()
    cprev_d = nc.dram_tensor("c_prev", [BLOC, H, 1], f32, kind="ExternalInput").ap()
    A_d = nc.dram_tensor("A", [BLOC, H, H], f32, kind="ExternalInput").ap()
    At_d = nc.dram_tensor("At_prev", [BLOC, H, H], f32, kind="ExternalInput").ap()
    W1_d = nc.dram_tensor("W1", [3 * H, H], f32, kind="ExternalInput").ap()
    b1_d = nc.dram_tensor("b1", [3 * H], f32, kind="ExternalInput").ap()
    W2_d = nc.dram_tensor("W2", [H, 3 * H], f32, kind="ExternalInput").ap()
    b2_d = nc.dram_tensor("b2", [H], f32, kind="ExternalInput").ap()

    hnew_d = nc.dram_tensor("h_new", [BLOC, H], f32, kind="ExternalOutput").ap()
    cnew_d = nc.dram_tensor("c_new", [BLOC, H, H], f32, kind="ExternalOutput").ap()
    At_o_d = nc.dram_tensor("A_t", [BLOC, H, H], f32, kind="ExternalOutput").ap()

    with tile.TileContext(nc) as tc, ExitStack() as ctx:
        const = ctx.enter_context(tc.tile_pool(name="const", bufs=1))
        wpool = ctx.enter_context(tc.tile_pool(name="wpool", bufs=1))
        work = ctx.enter_context(tc.tile_pool(name="work", bufs=2))
        ptr = ctx.enter_context(tc.tile_pool(name="ptr", bufs=2, space="PSUM"))
        pmm1 = ctx.enter_context(tc.tile_pool(name="pmm1", bufs=2, space="PSUM"))
        pmm2 = ctx.enter_context(tc.tile_pool(name="pmm2", bufs=4, space="PSUM"))

        # ------- tiny inputs first (gates feed the innov critical path)
        g_all = const.tile([128, BLOC, 2, IB], f32, tag="g_all")
        nc.sync.dma_start(
            out=g_all,
            in_=gates_d.rearrange("b (zm ib p) -> p b zm ib", zm=2, ib=IB, p=128),
        )
        c_sb = const.tile([128, BLOC, IB], f32, tag="c_sb")
        nc.sync.dma_start(
            out=c_sb,
            in_=cprev_d.rearrange("b (ib p) one -> p b (ib one)", ib=IB, p=128),
        )
        b1_sb = const.tile([128, KT], f32, tag="b1_sb")
        nc.sync.dma_start(out=b1_sb, in_=b1_d.rearrange("(kt p) -> p kt", p=128))
        # batch-0 At_prev load next (startup critical path)
        at0 = work.tile([128, IB, H], f32, tag="at", bufs=2, name="at0")
        nc.sync.dma_start(
            out=at0, in_=At_d[0].rearrange("(ib p) j -> p ib j", p=128)
        )
        z_all = const.tile([128, BLOC, IB], f32, tag="z_all")
        nc.scalar.activation(out=z_all, in_=g_all[:, :, 0, :], func=AF.Tanh)
        m_all = const.tile([128, BLOC, IB], f32, tag="m_all")
        nc.scalar.activation(out=m_all, in_=g_all[:, :, 1, :], func=AF.Sigmoid)
        # ------- whole-W loads, split across SP and ACT HW queues
        w1n = const.tile([128, KT, H], f32, tag="w1n")
        for g in range(3):
            nc.sync.dma_start(
                out=w1n[:, g * 4 : (g + 1) * 4, :],
                in_=W1_d.rearrange("(kt p) j -> p kt j", p=128)[
                    :, g * 4 : (g + 1) * 4, :
                ],
            )
        w2n = const.tile([128, IB, 3 * H], f32, tag="w2n")
        for g in range(2):
            nc.sync.dma_start(
                out=w2n[:, g * 2 : (g + 1) * 2, :],
                in_=W2_d.rearrange("(hb p) k -> p hb k", p=128)[
                    :, g * 2 : (g + 1) * 2, :
                ],
            )

        # ---------------- constants / small inputs ----------------
        ident_f = const.tile([128, 128], f32, tag="ident_f")
        make_identity(nc, ident_f)
        ident_b = const.tile([128, 128], bf16, tag="ident_b")
        make_identity(nc, ident_b)

        # identity slabs for (I - K_t*M): ones at (p, ib*128+p)
        I_tiles = []
        for ib in range(IB):
            it = const.tile([128, H], f32, tag=f"I{ib}")
            nc.gpsimd.memset(it, 0.0)
            nc.gpsimd.affine_select(
                out=it,
                in_=it,
                compare_op=OP.not_equal,
                fill=1.0,
                base=ib * 128,
                pattern=[[-1, H]],
                channel_multiplier=1,
            )
            I_tiles.append(it)

        # cmn = -c_prev*M ; mneg = -M ; m512 = M/512
        cmn = const.tile([128, BLOC, IB], f32, tag="cmn")
        nc.vector.scalar_tensor_tensor(
            out=cmn, in0=c_sb, scalar=-1.0, in1=m_all, op0=OP.mult, op1=OP.mult
        )
        mneg = const.tile([128, BLOC, IB], f32, tag="mneg")
        nc.vector.tensor_scalar_mul(mneg, m_all, -1.0)
        m512 = const.tile([128, BLOC, IB], f32, tag="m512")
        nc.vector.tensor_scalar_mul(m512, m_all, 1.0 / H)

        b2_sb = None
        if with_b2:
            b2_sb = const.tile([128, H], f32, tag="b2_sb")
            nc.sync.dma_start(out=b2_sb, in_=b2_d.to_broadcast((128, H)))

        h_tile = const.tile([128, BLOC, IB], f32, tag="h_tile")

        # ---------------- weight transposes (PE) ----------------
        w1t = [
            wpool.tile([128, 3 * H], bf16, tag=f"w1t{jc}", name=f"w1t{jc}")
            for jc in range(JC)
        ]
        w2t = [
            wpool.tile([128, H], bf16, tag=f"w2t{kt}", name=f"w2t{kt}")
            for kt in range(KT)
        ]

        def emit_w1_prep(g):
            for jc in range(JC):
                if True:
                    pt = pmm1.tile([128, 512], f32, tag="pmm1")
                    for q in range(4):
                        kt = g * 4 + q
                        nc.tensor.transpose(
                            out=pt[:, q * 128 : (q + 1) * 128],
                            in_=w1n[:, kt, jc * 128 : (jc + 1) * 128],
                            identity=ident_f,
                        )
                    nc.vector.tensor_copy(w1t[jc][:, g * 512 : (g + 1) * 512], pt)

        def emit_w2_prep():
            for kt in range(KT):
                pt = pmm1.tile([128, 512], f32, tag="pmm1")
                for hb in range(IB):
                    nc.tensor.transpose(
                        out=pt[:, hb * 128 : (hb + 1) * 128],
                        in_=w2n[:, hb, kt * 128 : (kt + 1) * 128],
                        identity=ident_f,
                    )
                nc.vector.tensor_copy(w2t[kt], pt)

        # ---------------- per-batch main loop ----------------
        def emit_innov(b, at_all):
            innov = []
            for ib in range(IB):
                inv = work.tile([128, H], bf16, tag="innov", bufs=8, name="inv")
                nc.vector.tensor_scalar(
                    out=inv,
                    in0=at_all[:, ib, :],
                    scalar1=cmn[:, b, ib : ib + 1],
                    scalar2=z_all[:, b, ib : ib + 1],
                    op0=OP.mult,
                    op1=OP.add,
                )
                innov.append(inv)
            return innov

        innov_next = emit_innov(0, at0)
        for b in range(BLOC):
            innov = innov_next

            # transpose innov -> innovT[jc] : [128(j), H(i)] bf16
            innovT = []
            for jc in range(JC):
                pt = ptr.tile([128, 512], bf16, tag="ptrb", bufs=2)
                for ib in range(IB):
                    nc.tensor.transpose(
                        out=pt[:, ib * 128 : (ib + 1) * 128],
                        in_=innov[ib][:, jc * 128 : (jc + 1) * 128],
                        identity=ident_b,
                    )
                iT = work.tile([128, H], bf16, tag="innovT", bufs=8)
                nc.vector.tensor_copy(iT, pt)
                innovT.append(iT)

            if b == 0:
                for g in range(KT // 4):
                    emit_w1_prep(g)

            # MM1: hiddenT[kt] = relu(W1T-block^T @ innovT + b1)
            ht = work.tile([128, KT, H], bf16, tag="ht")
            for kt in range(KT):
                ph = pmm1.tile([128, H], f32, tag="pmm1")
                for jc in range(JC):
                    nc.tensor.matmul(
                        ph,
                        lhsT=w1t[jc][:, kt * 128 : (kt + 1) * 128],
                        rhs=innovT[jc],
                        start=(jc == 0),
                        stop=(jc == JC - 1),
                    )
                nc.scalar.activation(
                    out=ht[:, kt, :],
                    in_=ph,
                    func=AF.Relu,
                    bias=b1_sb[:, kt : kt + 1],
                )

            if b == 0:
                emit_w2_prep()

            # prefetch + compute next batch's innov ahead of this batch's
            # epilogue so the DVE FIFO doesn't delay the next transposes
            if b + 1 < BLOC:
                at_n = work.tile([128, IB, H], f32, tag="at", bufs=2, name="at")
                nc.sync.dma_start(
                    out=at_n,
                    in_=At_d[b + 1].rearrange("(ib p) j -> p ib j", p=128),
                )
                innov_next = emit_innov(b + 1, at_n)

            # load A rows (needed in epilogue)
            a_all = work.tile([128, IB, H], f32, tag="arows", bufs=2)
            nc.sync.dma_start(
                out=a_all, in_=A_d[b].rearrange("(ib p) j -> p ib j", p=128)
            )

            cn_all = work.tile([128, IB, H], f32, tag="cn", bufs=2)
            atout_all = work.tile([128, IB, H], f32, tag="atout", bufs=2)

            # MM2 + epilogue per row-block
            for ib in range(IB):
                pk = pmm2.tile([128, H], f32, tag="pmm2")
                for kt in range(KT):
                    nc.tensor.matmul(
                        pk,
                        lhsT=ht[:, kt, ib * 128 : (ib + 1) * 128],
                        rhs=w2t[kt],
                        start=(kt == 0),
                        stop=(kt == KT - 1),
                    )

                kt_src = pk
                if with_b2:
                    kb = work.tile([128, H], f32, tag="kb2", bufs=2)
                    nc.vector.tensor_tensor(out=kb, in0=pk, in1=b2_sb, op=OP.add)
                    kt_src = kb

                # sarg = (K_t * -M) + I
                sarg = work.tile([128, H], f32, tag="sarg", bufs=2)
                nc.vector.scalar_tensor_tensor(
                    out=sarg,
                    in0=kt_src,
                    scalar=mneg[:, b, ib : ib + 1],
                    in1=I_tiles[ib],
                    op0=OP.mult,
                    op1=OP.add,
                )
                sig = work.tile([128, H], f32, tag="sig", bufs=2)
                nc.scalar.activation(out=sig, in_=sarg, func=AF.Sigmoid)
                # tkz = K_t * z
                tkz = work.tile([128, H], f32, tag="tkz", bufs=2)
                nc.scalar.activation(
                    out=tkz,
                    in_=kt_src,
                    func=AF.Copy,
                    scale=z_all[:, b, ib : ib + 1],
                )
                # A_t = sig * A
                nc.vector.tensor_tensor(
                    out=atout_all[:, ib, :],
                    in0=sig,
                    in1=a_all[:, ib, :],
                    op=OP.mult,
                )
                # c_new = A_t*c_prev + tkz  (+ fused row-sum)
                rs = work.tile([128, 1], f32, tag="rs", bufs=4)
                nc.vector.scalar_tensor_tensor(
                    out=cn_all[:, ib, :],
                    in0=atout_all[:, ib, :],
                    scalar=c_sb[:, b, ib : ib + 1],
                    in1=tkz,
                    op0=OP.mult,
                    op1=OP.add,
                    accum_out=rs,
                )
                # h_new = rowsum * M/512
                nc.vector.tensor_scalar(
                    out=h_tile[:, b, ib : ib + 1],
                    in0=rs,
                    scalar1=m512[:, b, ib : ib + 1],
                    scalar2=None,
                    op0=OP.mult,
                )
                if b == BLOC - 1:
                    # last batch: per-row-block stores shorten the tail
                    nc.sync.dma_start(
                        out=At_o_d[b, ib * 128 : (ib + 1) * 128, :],
                        in_=atout_all[:, ib, :],
                    )
                    nc.sync.dma_start(
                        out=cnew_d[b, ib * 128 : (ib + 1) * 128, :],
                        in_=cn_all[:, ib, :],
                    )

            if b < BLOC - 1:
                nc.gpsimd.dma_start(
                    out=At_o_d[b].rearrange("(ib p) j -> p ib j", p=128),
                    in_=atout_all,
                )
                nc.gpsimd.dma_start(
                    out=cnew_d[b].rearrange("(ib p) j -> p ib j", p=128),
                    in_=cn_all,
                )

        nc.sync.dma_start(
            out=hnew_d.rearrange("b (ib p) -> p b ib", ib=IB, p=128), in_=h_tile
        )

    nc.compile()
    return nc


def _get_program(with_b2):
    key = ("prog", bool(with_b2))
    if key not in _CACHE:
        _CACHE[key] = _build_program(with_b2)
    return _CACHE[key]


def kernel(**inputs):
    from concourse.bass_utils import run_bass_kernel_spmd

    gates = np.ascontiguousarray(np.asarray(inputs["gates"], dtype=np.float32))
    c_prev = np.ascontiguousarray(np.asarray(inputs["c_prev"], dtype=np.float32))
    A = np.ascontiguousarray(np.asarray(inputs["A"], dtype=np.float32))
    At_prev = np.ascontiguousarray(np.asarray(inputs["At_prev"], dtype=np.float32))
    W1 = np.ascontiguousarray(np.asarray(inputs["W1"], dtype=np.float32))
    b1 = np.ascontiguousarray(np.asarray(inputs["b1"], dtype=np.float32))
    W2 = np.ascontiguousarray(np.asarray(inputs["W2"], dtype=np.float32))
    b2 = np.ascontiguousarray(np.asarray(inputs["b2"], dtype=np.float32))

    with_b2 = bool(np.any(b2))
    nc = _get_program(with_b2)

    in_maps = []
    for c in range(NCORES):
        s = slice(c * BLOC, (c + 1) * BLOC)
        in_maps.append(
            {
                "gates": gates[s],
                "c_prev": c_prev[s],
                "A": A[s],
                "At_prev": At_prev[s],
                "W1": W1,
                "b1": b1,
                "W2": W2,
                "b2": b2,
            }
        )

    res = run_bass_kernel_spmd(nc, in_maps, core_ids=list(range(NCORES)))
    h_new = np.concatenate([res.results[c]["h_new"] for c in range(NCORES)], axis=0)
    c_new = np.concatenate([res.results[c]["c_new"] for c in range(NCORES)], axis=0)
    A_t = np.concatenate([res.results[c]["A_t"] for c in range(NCORES)], axis=0)
    return (
        h_new.astype(np.float32),
        c_new.astype(np.float32),
        A_t.astype(np.float32),
    )
